# revision 3
# baseline (speedup 1.0000x reference)
"""Trainium2 Bass kernel for nn_Mlp_62603443306826 (NeuMF + ragged masked-mean MLP).

Sharding: pure data-parallel over the batch dim (1024 -> 8 cores x 128 samples).
Small weights / NCF embedding gathers are replicated. BatchNorm runs in
training mode (batch statistics over the full 1024-sample batch), so the
per-core partial sums / sums-of-squares of the fc1 output are combined with a
tiny [128, 8] AllReduce before normalization.

Device pipeline per core:
  1. masked mean over user (200) / hashtag (50) positions: fused
     multiply-accumulate on the vector engine, one op per position,
     mask pre-scaled by 1/len on host  -> acc_u/acc_h [128, 768] (fp32 exact)
  2. NeuMF tower (16->32->16->8 MLP, transposed activations) on PE + ACT
  3. PE transposes acc -> x.T chunks; fc1 as 4x13 accumulating matmuls
     (output transposed: hidden on partitions)
  4. BN stats (sum / sum-of-squares per hidden unit) -> AllReduce -> fused
     scale/shift+relu via one activation per chunk; fc3 matmul; sigmoid.

fc1_b is intentionally not applied: training-mode BatchNorm subtracts the
batch mean, which cancels any constant bias added after fc1 exactly.
"""

import os
import sys

for _p in ("/opt/trn_rl_repo", "/root/.axon_site/_ro/trn_rl_repo"):
    if os.path.isdir(_p) and _p not in sys.path:
        sys.path.append(_p)

import numpy as np

import concourse.bacc as bacc
import concourse.tile as tile
import concourse.mybir as mybir
from concourse.bass_utils import run_bass_kernel_spmd

N_CORES = 8
B = 1024
S = B // N_CORES  # 128 samples per core
LU, LH, D = 200, 50, 768
HID = 512
EPS = 1e-5
F32 = mybir.dt.float32

UCHUNK = 8   # user positions per DMA chunk (25 chunks)
HCHUNK = 5   # hashtag positions per DMA chunk (10 chunks)

_nc_cache = None
last_result = None  # BassKernelResults of the most recent run (for test harness)


def _build_program():
    nc = bacc.Bacc("TRN2", target_bir_lowering=False, debug=False,
                   num_devices=N_CORES)

    def din(name, shape):
        return nc.dram_tensor(name, list(shape), F32, kind="ExternalInput").ap()

    uf = din("uf", (S, LU, D))
    hf = din("hf", (S, LH, D))
    um = din("um", (S, LU))
    hm = din("hm", (S, LH))
    h0T = din("h0T", (16, S))
    umfT = din("umfT", (8, S))
    imfT = din("imfT", (8, S))
    w0 = din("w0", (16, 32))
    b0 = din("b0", (32, 1))
    w1 = din("w1", (32, 16))
    b1 = din("b1", (16, 1))
    w2 = din("w2", (16, 8))
    b2 = din("b2", (8, 1))
    fc1w = din("fc1w", (1552, HID))
    gamma4 = din("gamma4", (S, 4))
    beta4 = din("beta4", (S, 4))
    fc3w4 = din("fc3w4", (S, 4))
    fc3b = din("fc3b", (S, 1))
    ident = din("ident", (S, S))
    out = nc.dram_tensor("out", [S, 1], F32, kind="ExternalOutput").ap()

    MUL = mybir.AluOpType.mult
    ADD = mybir.AluOpType.add
    SUB = mybir.AluOpType.subtract
    AF = mybir.ActivationFunctionType

    with tile.TileContext(nc) as tc:
        with (
            tc.tile_pool(name="consts", bufs=1) as pc,
            tc.tile_pool(name="featu", bufs=3) as pfu,
            tc.tile_pool(name="feath", bufs=3) as pfh,
            tc.tile_pool(name="acc", bufs=1) as pacc,
            tc.tile_pool(name="work", bufs=1) as pw,
            tc.tile_pool(name="scratch", bufs=2) as psc,
            tc.tile_pool(name="ps_t", bufs=2, space="PSUM") as ps_t,
            tc.tile_pool(name="ps_mm", bufs=2, space="PSUM") as ps_mm,
            tc.tile_pool(name="ps_ncf", bufs=2, space="PSUM") as ps_ncf,
            tc.tile_pool(name="ps_y", bufs=1, space="PSUM") as ps_y,
            tc.tile_pool(name="dram", bufs=1, space="DRAM") as pdram,
        ):
            # ---- constant / weight loads -------------------------------
            um_sb = pc.tile([S, LU], F32)
            hm_sb = pc.tile([S, LH], F32)
            nc.sync.dma_start(um_sb[:], um[:])
            nc.sync.dma_start(hm_sb[:], hm[:])

            fc1w_sb = []
            for c in range(13):
                k = 128 if c < 12 else 16
                t = pc.tile([k, HID], F32, tag=f"fc1w{c}")
                nc.sync.dma_start(t[:], fc1w[c * 128 : c * 128 + k, :])
                fc1w_sb.append(t)

            h0T_sb = pc.tile([16, S], F32)
            umfT_sb = pc.tile([8, S], F32)
            imfT_sb = pc.tile([8, S], F32)
            w0_sb = pc.tile([16, 32], F32)
            b0_sb = pc.tile([32, 1], F32)
            w1_sb = pc.tile([32, 16], F32)
            b1_sb = pc.tile([16, 1], F32)
            w2_sb = pc.tile([16, 8], F32)
            b2_sb = pc.tile([8, 1], F32)
            gamma_sb = pc.tile([S, 4], F32)
            beta_sb = pc.tile([S, 4], F32)
            fc3w_sb = pc.tile([S, 4], F32)
            fc3b_sb = pc.tile([S, 1], F32)
            ident_sb = pc.tile([S, S], F32)
            for t, src in (
                (h0T_sb, h0T), (umfT_sb, umfT), (imfT_sb, imfT),
                (w0_sb, w0), (b0_sb, b0), (w1_sb, w1), (b1_sb, b1),
                (w2_sb, w2), (b2_sb, b2), (gamma_sb, gamma4),
                (beta_sb, beta4), (fc3w_sb, fc3w4), (fc3b_sb, fc3b),
                (ident_sb, ident),
            ):
                nc.sync.dma_start(t[:], src[:])

            # ---- NeuMF tower (transposed activations) ------------------
            # runs early on PE/ACT while the masked-mean phase owns DVE/DMA
            ncfT = pw.tile([16, S], F32)
            p0 = ps_ncf.tile([32, S], F32, tag="ncf")
            nc.tensor.matmul(p0[:], w0_sb[:], h0T_sb[:], start=True, stop=True)
            h1T = pw.tile([32, S], F32)
            nc.scalar.activation(h1T[:], p0[:], AF.Relu, bias=b0_sb[:, 0:1])
            p1 = ps_ncf.tile([16, S], F32, tag="ncf")
            nc.tensor.matmul(p1[:], w1_sb[:], h1T[:], start=True, stop=True)
            h2T = pw.tile([16, S], F32)
            nc.scalar.activation(h2T[:], p1[:], AF.Relu, bias=b1_sb[:, 0:1])
            p2 = ps_ncf.tile([8, S], F32, tag="ncf")
            nc.tensor.matmul(p2[:], w2_sb[:], h2T[:], start=True, stop=True)
            nc.scalar.activation(ncfT[0:8, :], p2[:], AF.Relu, bias=b2_sb[:, 0:1])
            # engines can't address a partition base of 8; compute the mf
            # product at base 0 and DMA it into rows 8:16
            mfT = pw.tile([8, S], F32)
            nc.vector.tensor_tensor(mfT[:], umfT_sb[:], imfT_sb[:], op=MUL)
            nc.sync.dma_start(ncfT[8:16, :], mfT[:])

            # ---- masked mean: acc[s, d] = sum_l feats[s, l, d] * mask[s, l]
            acc_u = pacc.tile([S, D], F32)
            acc_h = pacc.tile([S, D], F32)
            for (feats, mask_sb, L, CH, acc) in (
                (uf, um_sb, LU, UCHUNK, acc_u),
                (hf, hm_sb, LH, HCHUNK, acc_h),
            ):
                for l0 in range(0, L, CH):
                    ft = (pfu if acc is acc_u else pfh).tile([S, CH, D], F32)
                    nc.sync.dma_start(ft[:], feats[:, l0 : l0 + CH, :])
                    for li in range(CH):
                        l = l0 + li
                        if l == 0:
                            nc.vector.tensor_scalar_mul(
                                acc[:], ft[:, li, :], mask_sb[:, l : l + 1])
                        else:
                            nc.vector.scalar_tensor_tensor(
                                acc[:], ft[:, li, :], mask_sb[:, l : l + 1],
                                acc[:], op0=MUL, op1=ADD)

            # ---- transpose embeddings into x.T chunks ------------------
            xT = pw.tile([S, 12 * S], F32)
            for c in range(12):
                src = acc_u if c < 6 else acc_h
                off = (c % 6) * S
                pt = ps_t.tile([S, S], F32, tag="tr")
                nc.tensor.matmul(pt[:], src[:, off : off + S], ident_sb[:],
                                 is_transpose=True)
                nc.vector.tensor_copy(xT[:, c * S : (c + 1) * S], pt[:])

            # ---- fc1 (output transposed: hidden on partitions) ---------
            x1 = pw.tile([S, HID], F32)
            stats = pw.tile([S, 8], F32)
            sq_scr = psc.tile([S, S], F32, tag="sq")
            for m in range(4):
                pm = ps_mm.tile([S, S], F32, tag="fc1")
                for c in range(13):
                    rhs = xT[:, c * S : (c + 1) * S] if c < 12 else ncfT[:]
                    nc.tensor.matmul(
                        pm[:], fc1w_sb[c][:, m * 128 : (m + 1) * 128], rhs,
                        start=(c == 0), stop=(c == 12))
                nc.vector.tensor_copy(x1[:, m * 128 : (m + 1) * 128], pm[:])
                nc.vector.tensor_reduce(
                    stats[:, m : m + 1], x1[:, m * 128 : (m + 1) * 128],
                    axis=mybir.AxisListType.X, op=ADD)
                sq_scr = psc.tile([S, S], F32, tag="sq")
                nc.scalar.activation(
                    sq_scr[:], x1[:, m * 128 : (m + 1) * 128], AF.Square,
                    accum_out=stats[:, 4 + m : 5 + m])

            # ---- AllReduce batch stats over the 8 cores ----------------
            cc_in = pdram.tile([S, 8], F32)
            cc_out = pdram.tile([S, 8], F32)
            nc.sync.dma_start(cc_in[:], stats[:])
            nc.gpsimd.collective_compute(
                "AllReduce", ADD,
                replica_groups=[list(range(N_CORES))],
                ins=[cc_in.opt()], outs=[cc_out.opt()])
            red = pw.tile([S, 8], F32)
            nc.sync.dma_start(red[:], cc_out[:])

            # ---- BN coefficients: A = gamma*rsqrt(var+eps), B = beta-mu*A
            mm8 = pw.tile([S, 8], F32)
            nc.vector.tensor_scalar_mul(mm8[:], red[:], 1.0 / B)
            var4 = pw.tile([S, 4], F32)
            nc.vector.tensor_tensor(var4[:], mm8[:, 0:4], mm8[:, 0:4], op=MUL)
            nc.vector.tensor_tensor(var4[:], mm8[:, 4:8], var4[:], op=SUB)
            nc.vector.tensor_scalar_add(var4[:], var4[:], EPS)
            std4 = pw.tile([S, 4], F32)
            nc.scalar.activation(std4[:], var4[:], AF.Sqrt, bias=0.0)
            ab = pw.tile([S, 8], F32)
            nc.vector.reciprocal(ab[:, 0:4], std4[:])
            nc.vector.tensor_tensor(ab[:, 0:4], gamma_sb[:], ab[:, 0:4], op=MUL)
            nc.vector.tensor_tensor(ab[:, 4:8], mm8[:, 0:4], ab[:, 0:4], op=MUL)
            nc.vector.tensor_tensor(ab[:, 4:8], beta_sb[:], ab[:, 4:8], op=SUB)

            # ---- BN + relu fused, then fc3 + sigmoid -------------------
            rT = pw.tile([S, HID], F32)
            for m in range(4):
                nc.scalar.activation(
                    rT[:, m * 128 : (m + 1) * 128],
                    x1[:, m * 128 : (m + 1) * 128], AF.Relu,
                    bias=ab[:, 4 + m : 5 + m], scale=ab[:, m : m + 1])
            py = ps_y.tile([S, 1], F32)
            for m in range(4):
                nc.tensor.matmul(py[:], rT[:, m * 128 : (m + 1) * 128],
                                 fc3w_sb[:, m : m + 1],
                                 start=(m == 0), stop=(m == 3))
            out_sb = pw.tile([S, 1], F32)
            nc.scalar.activation(out_sb[:], py[:], AF.Sigmoid, bias=fc3b_sb[:, 0:1])
            nc.sync.dma_start(out[:], out_sb[:])

    nc.compile()
    return nc


def kernel(**inputs) -> np.ndarray:
    global _nc_cache, last_result
    uf = np.asarray(inputs["user_features"], np.float32)
    hf = np.asarray(inputs["hashtag_features"], np.float32)
    ul = np.asarray(inputs["user_lens"])
    hl = np.asarray(inputs["hashtag_lens"])
    users = np.asarray(inputs["users"])
    items = np.asarray(inputs["items"])

    um = ((np.arange(LU)[None, :] < ul[:, None]) / ul[:, None]).astype(np.float32)
    hm = ((np.arange(LH)[None, :] < hl[:, None]) / hl[:, None]).astype(np.float32)

    h0 = np.concatenate(
        [np.asarray(inputs["u_mlp"])[users], np.asarray(inputs["i_mlp"])[items]], axis=1
    ).astype(np.float32)  # [B, 16]
    umf = np.asarray(inputs["u_mf"])[users].astype(np.float32)  # [B, 8]
    imf = np.asarray(inputs["i_mf"])[items].astype(np.float32)

    C = np.ascontiguousarray
    rep = {
        "w0": C(np.asarray(inputs["mlp_w0"], np.float32)),
        "b0": C(np.asarray(inputs["mlp_b0"], np.float32).reshape(32, 1)),
        "w1": C(np.asarray(inputs["mlp_w1"], np.float32)),
        "b1": C(np.asarray(inputs["mlp_b1"], np.float32).reshape(16, 1)),
        "w2": C(np.asarray(inputs["mlp_w2"], np.float32)),
        "b2": C(np.asarray(inputs["mlp_b2"], np.float32).reshape(8, 1)),
        "fc1w": C(np.asarray(inputs["fc1_w"], np.float32)),
        "gamma4": C(np.asarray(inputs["bn_gamma"], np.float32).reshape(4, 128).T),
        "beta4": C(np.asarray(inputs["bn_beta"], np.float32).reshape(4, 128).T),
        "fc3w4": C(np.asarray(inputs["fc3_w"], np.float32).reshape(4, 128).T),
        "fc3b": np.full((S, 1), np.float32(np.asarray(inputs["fc3_b"]).reshape(-1)[0])),
        "ident": np.eye(S, dtype=np.float32),
    }

    in_maps = []
    for c in range(N_CORES):
        sl = slice(c * S, (c + 1) * S)
        m = {
            "uf": C(uf[sl]), "hf": C(hf[sl]),
            "um": C(um[sl]), "hm": C(hm[sl]),
            "h0T": C(h0[sl].T), "umfT": C(umf[sl].T), "imfT": C(imf[sl].T),
        }
        m.update(rep)
        in_maps.append(m)

    if _nc_cache is None:
        _nc_cache = _build_program()
    res = run_bass_kernel_spmd(_nc_cache, in_maps, core_ids=list(range(N_CORES)))
    last_result = res
    return np.concatenate([res.results[c]["out"] for c in range(N_CORES)], axis=0)
